# revision 19
# baseline (speedup 1.0000x reference)
"""Trainium2 Bass kernel: 3x3 same-padding conv2d, 64->64 channels, on
x(16,64,112,112) f32, data-parallel over batch across 8 NeuronCores.

Strategy (per core, 2 images):
  - Host pre-pads each image to 114x114 (zeros) so the input DMA is one
    fully-contiguous [128, 114*114] bf16 transfer (partitions 0-63 =
    image0 cin, 64-127 = image1 cin); every conv tap is then a flat
    offset slice of the SBUF tile.
  - Conv = 9 accumulated matmuls (one per tap) with K=cin=64, M=cout=64,
    N=456 (4 output rows x 114). PE-array quadrant packing via
    tile_position: 4 independent 64x64 matmuls run concurrently
    (2 images x 2 adjacent row-blocks), bf16 operands, fp32 PSUM.
  - PSUM -> SBUF drain fused with bias add (alternating scalar/vector
    engines) into two big staging tiles, drained to HBM in quarter
    chunks so output DMA overlaps compute.
"""

import numpy as np
import ml_dtypes

import concourse.bacc as bacc
import concourse.mybir as mybir
import concourse.tile as tile
from concourse import bass_utils

FP32 = mybir.dt.float32
BF16 = mybir.dt.bfloat16

P = 128          # SBUF partitions
CIN = 64
COUT = 64
H = W = 112
Wp = W + 2       # padded width
Hp = H + 2
NROW = 4         # output rows per matmul block
NBLK = NROW * Wp  # matmul free size = 456
G = 14           # row-block pairs (8 rows per group)
XS_LEN = Hp * Wp + 4   # 12996 + slack for tap-offset overrun
OUT_LEN = G * NBLK     # 6384 per half

TAPS = [(kh, kw) for kh in range(3) for kw in range(3)]
# output DMA chunks: drain every 2 finished groups, per-group at the tail
QUARTER_END = {1: (0, 2), 3: (2, 4), 5: (4, 6), 7: (6, 8), 9: (8, 10),
               11: (10, 12), 12: (12, 13), 13: (13, 14)}


def _build_nc(n_cores: int = 8):
    nc = bacc.Bacc("TRN2", target_bir_lowering=False, debug=False,
                   num_devices=n_cores)
    x_d = nc.dram_tensor("xin", (P, XS_LEN), BF16, kind="ExternalInput").ap()
    w_d = nc.dram_tensor("wt", (P, 9 * COUT), BF16, kind="ExternalInput").ap()
    b_d = nc.dram_tensor("bias", (P, 1), FP32, kind="ExternalInput").ap()
    y_d = nc.dram_tensor("yout", (2, P, OUT_LEN), FP32,
                         kind="ExternalOutput").ap()

    with tile.TileContext(nc) as tc:
        with tc.tile_pool(name="main", bufs=1) as pool, \
             tc.tile_pool(name="psum", bufs=4, space="PSUM") as psum_pool:
            xs = pool.tile([P, XS_LEN], BF16, name="xs")
            wsb = pool.tile([P, 9 * COUT], BF16, name="wsb")
            bsb = pool.tile([P, 1], FP32, name="bsb")
            osbA = pool.tile([P, OUT_LEN], FP32, name="osbA")
            osbB = pool.tile([P, OUT_LEN], FP32, name="osbB")

            # Weights/bias on the scalar HWDGE ring so the sync ring's first
            # descriptor is input chunk 0 (input feeds the PE critical path).
            nc.scalar.dma_start(wsb[:, :], w_d[:, :])
            nc.scalar.dma_start(bsb[:, :], b_d[:, :])

            # Input: contiguous on both sides; graduated chunks — small first
            # chunk un-gates group 0 fast, big later chunks for DMA
            # efficiency; alternate the two HWDGE rings to balance load.
            bounds = [0, 1254, 3306, 8151, XS_LEN]
            for c0, c1 in zip(bounds, bounds[1:]):
                nc.sync.dma_start(xs[:, c0:c1], x_d[:, c0:c1])

            # Tap-outer over chunks of 3 groups: one LDWEIGHTS set per tap
            # covers 3 group-slots (~570ns of matmul streaming), hiding the
            # weight-load latency that otherwise paces the PE.
            CHUNKS = [(0, 3), (3, 6), (6, 9), (9, 12), (12, 14)]
            for c0, c1 in CHUNKS:
                ps = {}
                for g in range(c0, c1):
                    ps[g] = (
                        psum_pool.tile([P, NBLK], FP32, tag=f"psA{g % 2}",
                                       bufs=2, name=f"psA_{g}"),
                        psum_pool.tile([P, NBLK], FP32, tag=f"psB{g % 2}",
                                       bufs=2, name=f"psB_{g}"),
                    )
                for t, (kh, kw) in enumerate(TAPS):
                    st = t == 0
                    sp = t == 8
                    w0 = wsb[0:64, t * 64:(t + 1) * 64]
                    w1 = wsb[64:128, t * 64:(t + 1) * 64]
                    for g in range(c0, c1):
                        psA, psB = ps[g]
                        oA = (8 * g + kh) * Wp + kw
                        oB = (8 * g + 4 + kh) * Wp + kw
                        # 4 concurrent PE-quadrant matmuls: (row_grp, col_grp)
                        nc.tensor.matmul(psA[0:64, :], w0,
                                         xs[0:64, oA:oA + NBLK],
                                         start=st, stop=sp,
                                         tile_position=(0, 0))
                        nc.tensor.matmul(psA[64:128, :], w1,
                                         xs[64:128, oA:oA + NBLK],
                                         start=st, stop=sp,
                                         tile_position=(64, 64))
                        nc.tensor.matmul(psB[0:64, :], w1,
                                         xs[64:128, oB:oB + NBLK],
                                         start=st, stop=sp,
                                         tile_position=(64, 0))
                        nc.tensor.matmul(psB[64:128, :], w0,
                                         xs[0:64, oB:oB + NBLK],
                                         start=st, stop=sp,
                                         tile_position=(0, 64))
                for g in range(c0, c1):
                    psA, psB = ps[g]
                    dstA = osbA[:, g * NBLK:(g + 1) * NBLK]
                    dstB = osbB[:, g * NBLK:(g + 1) * NBLK]
                    # PSUM -> SBUF drain with fused bias add, split engines
                    if g % 2 == 0:
                        nc.scalar.add(dstA, psA[:, :], bsb[:, 0:1])
                        nc.vector.tensor_scalar_add(dstB, psB[:, :],
                                                    bsb[:, 0:1])
                    else:
                        nc.vector.tensor_scalar_add(dstA, psA[:, :],
                                                    bsb[:, 0:1])
                        nc.scalar.add(dstB, psB[:, :], bsb[:, 0:1])
                # Drain this chunk's rows to HBM (overlaps next chunk's PE)
                s0, s1 = c0 * NBLK, c1 * NBLK
                nc.sync.dma_start(y_d[0, :, s0:s1], osbA[:, s0:s1])
                nc.scalar.dma_start(y_d[1, :, s0:s1], osbB[:, s0:s1])

    nc.compile()
    return nc


_NC = None


def _get_nc():
    global _NC
    if _NC is None:
        _NC = _build_nc()
    return _NC


def _prep_in_maps(x, weights, bias, n_cores=8):
    # lhsT per tap: wt[cin, t*64+cout] = weights[cout, cin, kh, kw],
    # replicated into both partition halves.
    tmp = np.ascontiguousarray(
        weights.astype(np.float32).transpose(2, 3, 1, 0)).reshape(9, CIN, COUT)
    wt = np.empty((P, 9 * COUT), ml_dtypes.bfloat16)
    wt[0:64] = tmp.transpose(1, 0, 2).reshape(CIN, 9 * COUT)
    wt[64:128] = wt[0:64]
    bs = np.tile(np.asarray(bias, np.float32), 2).reshape(P, 1)

    xb = np.asarray(x, np.float32).astype(ml_dtypes.bfloat16)
    # pre-padded layout: [core, 128, 114*114(+slack)] with zero borders
    xp = np.zeros((n_cores, P, XS_LEN), ml_dtypes.bfloat16)
    interior = xp[:, :, :Hp * Wp].reshape(n_cores, P, Hp, Wp)
    interior[:, :, 1:1 + H, 1:1 + W] = xb.reshape(n_cores, P, H, W)
    in_maps = []
    for i in range(n_cores):
        in_maps.append({"xin": xp[i], "wt": wt, "bias": bs})
    return in_maps


def _assemble(yout):
    # yout: [2, 128, 6384] -> (2, 64, 112, 112) for this core's two images.
    y = yout.reshape(2, 2, 64, G, NROW, Wp)[:, :, :, :, :, :W]
    out = np.empty((2, 64, G, 8, W), np.float32)
    out[0, :, :, 0:4] = y[0, 0]   # osbA[0:64]   = img0 rows 8g..8g+4
    out[1, :, :, 0:4] = y[0, 1]   # osbA[64:128] = img1 rows 8g..8g+4
    out[0, :, :, 4:8] = y[1, 1]   # osbB[64:128] = img0 rows 8g+4..8g+8
    out[1, :, :, 4:8] = y[1, 0]   # osbB[0:64]   = img1 rows 8g+4..8g+8
    return out.reshape(2, 64, H, W)


def kernel(x, weights, bias, _trace=False, _tmpdir=None):
    nc = _get_nc()
    in_maps = _prep_in_maps(x, weights, bias)
    res = bass_utils.run_bass_kernel_spmd(nc, in_maps,
                                          core_ids=list(range(8)),
                                          trace=_trace, tmpdir=_tmpdir)
    out = np.concatenate([_assemble(res.results[i]["yout"])
                          for i in range(8)], axis=0)
    if _trace:
        return out, res
    return out


# revision 20
# speedup vs baseline: 1.0857x; 1.0857x over previous
"""Trainium2 Bass kernel: 3x3 same-padding conv2d, 64->64 channels, on
x(16,64,112,112) f32, data-parallel over batch across 8 NeuronCores.

Strategy (per core, 2 images):
  - Host pre-pads each image to 114x114 (zeros) so the input DMA is one
    fully-contiguous [128, 114*114] bf16 transfer (partitions 0-63 =
    image0 cin, 64-127 = image1 cin); every conv tap is then a flat
    offset slice of the SBUF tile.
  - Conv = 9 accumulated matmuls (one per tap) with K=cin=64, M=cout=64,
    N=456 (4 output rows x 114). PE-array quadrant packing via
    tile_position: 4 independent 64x64 matmuls run concurrently
    (2 images x 2 adjacent row-blocks), bf16 operands, fp32 PSUM.
  - PSUM -> SBUF drain fused with bias add (alternating scalar/vector
    engines) into two big staging tiles, drained to HBM in quarter
    chunks so output DMA overlaps compute.
"""

import numpy as np
import ml_dtypes

import concourse.bacc as bacc
import concourse.mybir as mybir
import concourse.tile as tile
from concourse import bass_utils

FP32 = mybir.dt.float32
BF16 = mybir.dt.bfloat16

P = 128          # SBUF partitions
CIN = 64
COUT = 64
H = W = 112
Wp = W + 2       # padded width
Hp = H + 2
NROW = 4         # output rows per matmul block
NBLK = NROW * Wp  # matmul free size = 456
G = 14           # row-block pairs (8 rows per group)
XS_LEN = Hp * Wp + 4   # 12996 + slack for tap-offset overrun
OUT_LEN = G * NBLK     # 6384 per half

TAPS = [(kh, kw) for kh in range(3) for kw in range(3)]
# output DMA chunks: drain every 2 finished groups, per-group at the tail
QUARTER_END = {1: (0, 2), 3: (2, 4), 5: (4, 6), 7: (6, 8), 9: (8, 10),
               11: (10, 12), 12: (12, 13), 13: (13, 14)}


def _build_nc(n_cores: int = 8):
    nc = bacc.Bacc("TRN2", target_bir_lowering=False, debug=False,
                   num_devices=n_cores)
    x_d = nc.dram_tensor("xin", (P, XS_LEN), BF16, kind="ExternalInput").ap()
    w_d = nc.dram_tensor("wt", (P, 9 * COUT), BF16, kind="ExternalInput").ap()
    b_d = nc.dram_tensor("bias", (P, 1), FP32, kind="ExternalInput").ap()
    y_d = nc.dram_tensor("yout", (2, P, OUT_LEN), FP32,
                         kind="ExternalOutput").ap()

    with tile.TileContext(nc) as tc:
        with tc.tile_pool(name="main", bufs=1) as pool, \
             tc.tile_pool(name="psum", bufs=4, space="PSUM") as psum_pool:
            xs = pool.tile([P, XS_LEN], BF16, name="xs")
            wsb = pool.tile([P, 9 * COUT], BF16, name="wsb")
            bsb = pool.tile([P, 1], FP32, name="bsb")
            osbA = pool.tile([P, OUT_LEN], FP32, name="osbA")
            osbB = pool.tile([P, OUT_LEN], FP32, name="osbB")

            # Weights/bias on the scalar HWDGE ring so the sync ring's first
            # descriptor is input chunk 0 (input feeds the PE critical path).
            nc.scalar.dma_start(wsb[:, :], w_d[:, :])
            nc.scalar.dma_start(bsb[:, :], b_d[:, :])

            # Input: contiguous on both sides; graduated chunks — small first
            # chunk un-gates group 0 fast, big later chunks for DMA
            # efficiency; alternate the two HWDGE rings to balance load.
            bounds = [0, 1254, 3306, 8151, XS_LEN]
            for c0, c1 in zip(bounds, bounds[1:]):
                nc.sync.dma_start(xs[:, c0:c1], x_d[:, c0:c1])

            for g in range(G):
                psA = psum_pool.tile([P, NBLK], FP32, tag="psA", bufs=4)
                psB = psum_pool.tile([P, NBLK], FP32, tag="psB", bufs=4)
                rA = 8 * g
                rB = 8 * g + 4
                for t, (kh, kw) in enumerate(TAPS):
                    st = t == 0
                    sp = t == 8
                    w0 = wsb[0:64, t * 64:(t + 1) * 64]
                    w1 = wsb[64:128, t * 64:(t + 1) * 64]
                    oA = (rA + kh) * Wp + kw
                    oB = (rB + kh) * Wp + kw
                    # 4 concurrent PE-quadrant matmuls: (row_grp, col_grp)
                    nc.tensor.matmul(psA[0:64, :], w0,
                                     xs[0:64, oA:oA + NBLK],
                                     start=st, stop=sp, tile_position=(0, 0))
                    nc.tensor.matmul(psA[64:128, :], w1,
                                     xs[64:128, oA:oA + NBLK],
                                     start=st, stop=sp, tile_position=(64, 64))
                    nc.tensor.matmul(psB[0:64, :], w1,
                                     xs[64:128, oB:oB + NBLK],
                                     start=st, stop=sp, tile_position=(64, 0))
                    nc.tensor.matmul(psB[64:128, :], w0,
                                     xs[0:64, oB:oB + NBLK],
                                     start=st, stop=sp, tile_position=(0, 64))
                dstA = osbA[:, g * NBLK:(g + 1) * NBLK]
                dstB = osbB[:, g * NBLK:(g + 1) * NBLK]
                # PSUM -> SBUF drain with fused bias add, alternating engines
                if g % 2 == 0:
                    nc.scalar.add(dstA, psA[:, :], bsb[:, 0:1])
                    nc.vector.tensor_scalar_add(dstB, psB[:, :], bsb[:, 0:1])
                else:
                    nc.vector.tensor_scalar_add(dstA, psA[:, :], bsb[:, 0:1])
                    nc.scalar.add(dstB, psB[:, :], bsb[:, 0:1])
                # Drain finished chunks so output DMA overlaps compute
                if g in QUARTER_END:
                    g0, g1 = QUARTER_END[g]
                    s0, s1 = g0 * NBLK, g1 * NBLK
                    # split across the two HWDGE rings (SP + ACT)
                    nc.sync.dma_start(y_d[0, :, s0:s1], osbA[:, s0:s1])
                    nc.scalar.dma_start(y_d[1, :, s0:s1], osbB[:, s0:s1])

    nc.compile()
    return nc


_NC = None


def _get_nc():
    global _NC
    if _NC is None:
        _NC = _build_nc()
    return _NC


def _prep_in_maps(x, weights, bias, n_cores=8):
    # lhsT per tap: wt[cin, t*64+cout] = weights[cout, cin, kh, kw],
    # replicated into both partition halves.
    tmp = np.ascontiguousarray(
        weights.astype(np.float32).transpose(2, 3, 1, 0)).reshape(9, CIN, COUT)
    wt = np.empty((P, 9 * COUT), ml_dtypes.bfloat16)
    wt[0:64] = tmp.transpose(1, 0, 2).reshape(CIN, 9 * COUT)
    wt[64:128] = wt[0:64]
    bs = np.tile(np.asarray(bias, np.float32), 2).reshape(P, 1)

    xb = np.asarray(x, np.float32).astype(ml_dtypes.bfloat16)
    # pre-padded layout: [core, 128, 114*114(+slack)] with zero borders
    xp = np.zeros((n_cores, P, XS_LEN), ml_dtypes.bfloat16)
    interior = xp[:, :, :Hp * Wp].reshape(n_cores, P, Hp, Wp)
    interior[:, :, 1:1 + H, 1:1 + W] = xb.reshape(n_cores, P, H, W)
    in_maps = []
    for i in range(n_cores):
        in_maps.append({"xin": xp[i], "wt": wt, "bias": bs})
    return in_maps


def _assemble(yout):
    # yout: [2, 128, 6384] -> (2, 64, 112, 112) for this core's two images.
    y = yout.reshape(2, 2, 64, G, NROW, Wp)[:, :, :, :, :, :W]
    out = np.empty((2, 64, G, 8, W), np.float32)
    out[0, :, :, 0:4] = y[0, 0]   # osbA[0:64]   = img0 rows 8g..8g+4
    out[1, :, :, 0:4] = y[0, 1]   # osbA[64:128] = img1 rows 8g..8g+4
    out[0, :, :, 4:8] = y[1, 1]   # osbB[64:128] = img0 rows 8g+4..8g+8
    out[1, :, :, 4:8] = y[1, 0]   # osbB[0:64]   = img1 rows 8g+4..8g+8
    return out.reshape(2, 64, H, W)


def kernel(x, weights, bias, _trace=False, _tmpdir=None):
    nc = _get_nc()
    in_maps = _prep_in_maps(x, weights, bias)
    res = bass_utils.run_bass_kernel_spmd(nc, in_maps,
                                          core_ids=list(range(8)),
                                          trace=_trace, tmpdir=_tmpdir)
    out = np.concatenate([_assemble(res.results[i]["yout"])
                          for i in range(8)], axis=0)
    if _trace:
        return out, res
    return out
